# revision 37
# baseline (speedup 1.0000x reference)
"""Trainium2 Bass kernel for nn_ContrastiveLoss (8-core data-parallel).

Transposed-tile (G^T) design with cross-core pair deduplication:
  Each core c owns row-blocks {c, 8+c} (512 rows).  G = feats@feats.T is
  symmetric, so each unordered block-pair tile is computed by exactly one
  core.  Core c computes G^T column-tiles [j-rows, own-i] for 4 peers:
    peers c+1, c+2, c+3 -> raw fp8 ship (host: exp-sums/maxes).  The
        tiles for peers c-1, c-2, c-3 come from THOSE peers' ships
        (the host transposes).
    peer c+4 -> ACT exp -> bf16 e-tile; per-i e-sums via ones-matmul
        into an accumulating PSUM row (exact f32); per-i maxes via a
        DVE fold tree -> rmax.  (Pair (c, c+4) is exp'd by both ends.)
  The own-pair 512x512 tile (positives + diagonal) is computed exactly
  on the host (f64), which also sharpens the count margins.
  fp8 DoubleRow matmuls; PE also does the ones-matmul partition sums.
Host: negsum/thr/count/margins/possum (exact block-sum trick)/loss;
borderline rows (|G - thr| < MARGIN) recounted exactly in f32.
"""

import os
import sys

sys.path.insert(0, "/opt/trn_rl_repo")
os.environ["BASS_NEVER_TRACE"] = "1"

from contextlib import ExitStack

import numpy as np
import ml_dtypes

import concourse.mybir as mybir
import concourse.tile as tile
from concourse import bacc
from concourse.bass_utils import run_bass_kernel_spmd

TEMP = 0.02
OTHERWEIGHT = 0.5

NCORES = 8
N = 4096
F = 512
BS = 256
NCHUNK = 5            # own, then 4 peer chunks
PEER_OF_CHUNK = [0, 4, 1, 2, 3]   # chunk u holds blocks of peer c+PEER_OF_CHUNK[u]
EXP_CHUNK = 1         # chunk index treated with on-device exp (peer +4)
SHIP_CHUNKS = (2, 3, 4)           # fp8 raw ships (peers +1, +2, +3)
SHIP_SLOT = {2: 0, 3: 1, 4: 2}    # chunk -> ship_d row
# (chunk, half, engine) processing ring: consumers alternate ACT/DVE so the
# 3-deep PSUM slot rotation never stalls on one engine; the exp tile rides
# in the first input batch so its slow output chains finish mid-kernel
SCHED = [(1, 0, 'A'), (2, 0, 'D'), (1, 1, 'A'), (2, 1, 'A'),
         (3, 1, 'D'), (3, 0, 'A'), (4, 1, 'D'), (4, 0, 'A')]
ESB_ENGINE = 'A'
MARGIN = 14.0         # G-domain refinement margin (fp8 matmul + fp8 ship noise)
WARM_MMS = 10         # PE p-state warmup matmuls

F8 = ml_dtypes.float8_e4m3
BF16 = ml_dtypes.bfloat16

_BUILT = None
_LAST_RESULTS = None


def _build_nc():
    f32 = mybir.dt.float32
    bf16 = mybir.dt.bfloat16
    fp8 = mybir.dt.float8e4
    Exp = mybir.ActivationFunctionType.Exp
    Copy = mybir.ActivationFunctionType.Copy
    DR = mybir.MatmulPerfMode.DoubleRow
    MAX = mybir.AluOpType.max

    nc = bacc.Bacc("TRN2", target_bir_lowering=False, debug=False)
    # DRAM input laid out exactly like SBUF: [p, k*(NCHUNK*1024) + u*1024 + sx]
    ft_d = nc.dram_tensor("ft", [128, NCHUNK * 2048], fp8, kind="ExternalInput")
    ship_d = nc.dram_tensor("ship", [3, 128, 2048], fp8, kind="ExternalOutput")
    rmax_d = nc.dram_tensor("rmax", [128, 1024], bf16, kind="ExternalOutput")
    esum_d = nc.dram_tensor("esum", [1, 512], f32, kind="ExternalOutput")

    with tile.TileContext(nc) as tc, ExitStack() as ctx:
        ftp = ctx.enter_context(tc.tile_pool(name="ft", bufs=1))
        sp = ctx.enter_context(tc.tile_pool(name="sp", bufs=1))
        psp = ctx.enter_context(tc.tile_pool(name="ps", bufs=1, space="PSUM"))

        ft_t = ftp.tile([128, NCHUNK * 2048], fp8, name="ft", tag="ft")
        # view [p, k, u, s, x]: SBUF col = k*(NCHUNK*1024) + u*1024 + s*512 + x
        ftr = ft_t[:].rearrange("p (k u s x) -> p k u s x", k=2, u=NCHUNK, s=2)
        ftkr = ft_t[:].rearrange("p (k r) -> p k r", k=2)
        ftdr = ft_d.ap().rearrange("p (k r) -> p k r", k=2)
        # chunks 0+1 in one DMA (both needed before any matmul), rest singly
        for lo, hi in [(0, 2), (2, 3), (3, 4), (4, 5)]:
            nc.sync.dma_start(
                ftkr[:, :, lo * 1024:hi * 1024],
                ftdr[:, :, lo * 1024:hi * 1024],
            )

        eship = sp.tile([128, 3 * 2048], fp8, name="eship", tag="eship")
        et = sp.tile([128, 2048], bf16, name="et", tag="et")
        rfold = sp.tile([128, 1024], bf16, name="rfold", tag="rfold")
        esb = sp.tile([1, 512], f32, name="esb", tag="esb")
        ones_t = sp.tile([128, 1], bf16, name="ones", tag="ones")
        dummy = sp.tile([128, 1], bf16, name="dummy", tag="dummy")
        nc.vector.memset(ones_t[:], 1.0)

        # Hoist ACT table loads off the critical path: touch Exp and Copy
        # tables with tiny dummy activations before any real dependency.
        nc.scalar.activation(dummy[:], ones_t[:], Exp)
        nc.scalar.activation(dummy[:], ones_t[:], Copy)

        # PSUM: 3 rotating half-slots (6 banks) + esum row (1) + warm (1)
        ps_t = [psp.tile([128, 1024], f32, name=f"ps{i}", tag=f"ps{i}")
                for i in range(3)]
        es_ps = psp.tile([1, 512], f32, name="esps", tag="esps")
        warm_ps = psp.tile([128, 512], f32, name="warmps", tag="warmps")

        # PE p-state warmup: PE busy from t~0 so real matmuls hit full clock.
        warm = sp.tile([128, 256], fp8, name="warm", tag="warm")
        nc.vector.memset(warm[:], 0.0)
        wr = warm[:].rearrange("p (s x) -> p s x", s=2)
        for _ in range(WARM_MMS):
            nc.tensor.matmul(
                warm_ps[0:128, 0:128], wr[:, :, 0:128], wr[:, :, 0:128],
                start=True, stop=True, perf_mode=DR,
            )

        pend_ones = []    # deferred ones-MM slices (avoid PE head-of-line stall)
        ones_emitted = [0]

        def flush_ones():
            for sl in pend_ones:
                ones_emitted[0] += 1
                nc.tensor.matmul(
                    es_ps[:], ones_t[:], sl,
                    start=(ones_emitted[0] == 1),
                    stop=(ones_emitted[0] == 4),
                    skip_group_check=True,
                )
            pend_ones.clear()

        ship_halves_done = {u: 0 for u in SHIP_CHUNKS}
        for pos, (t, h, eng) in enumerate(SCHED):
            ps = ps_t[pos % 3]
            for g in (2 * h, 2 * h + 1):
                for k in range(2):
                    nc.tensor.matmul(
                        ps[:, (g % 2) * 512:(g % 2 + 1) * 512],
                        ftr[:, k, t, :, g * 128:(g + 1) * 128],
                        ftr[:, k, 0, :, :],
                        start=(k == 0),
                        stop=(k == 1),
                        perf_mode=DR,
                    )
            flush_ones()
            if t == EXP_CHUNK:
                with nc.allow_low_precision(reason="bf16 e; host-tol ok"):
                    nc.scalar.activation(
                        et[:, h * 1024:(h + 1) * 1024], ps[:], Exp, scale=TEMP)
                for g in (2 * h, 2 * h + 1):
                    pend_ones.append(et[:, g * 512:(g + 1) * 512])
                # per-half max fold (bf16 2x on DVE)
                with nc.allow_low_precision(reason="bf16 max; host refine"):
                    nc.vector.tensor_tensor(
                        rfold[:, h * 512:(h + 1) * 512],
                        et[:, h * 1024:h * 1024 + 512],
                        et[:, h * 1024 + 512:h * 1024 + 1024], op=MAX)
            else:
                slot = SHIP_SLOT[t]
                dst = eship[:, slot * 2048 + h * 1024:
                            slot * 2048 + (h + 1) * 1024]
                if eng == 'A':
                    nc.scalar.activation(dst, ps[:], Copy)
                elif eng == 'D':
                    nc.vector.tensor_copy(dst, ps[:])
                else:  # 'S': split the copy across both engines
                    nc.vector.tensor_copy(dst[:, 0:512], ps[:, 0:512])
                    nc.scalar.activation(dst[:, 512:1024], ps[:, 512:1024], Copy)
                ship_halves_done[t] += 1
                if t == 4:
                    # last ship tile: per-half DMA so the final transfer is small
                    nc.sync.dma_start(
                        ship_d.ap()[slot, :, h * 1024:(h + 1) * 1024], dst)
                elif ship_halves_done[t] == 2:
                    nc.sync.dma_start(
                        ship_d.ap()[slot],
                        eship[:, slot * 2048:(slot + 1) * 2048])
        flush_ones()
        if ESB_ENGINE == 'A':
            nc.scalar.activation(esb[:], es_ps[:], Copy)
        else:
            nc.vector.tensor_copy(esb[:], es_ps[:])
        # epilogue DMAs in expected-readiness order (HWDGE is in-order; a
        # slow-dep DMA issued early head-of-line-blocks everything after it)
        nc.sync.dma_start(rmax_d.ap()[:], rfold[:])
        nc.sync.dma_start(esum_d.ap()[:], esb[:])

    nc.compile()
    return nc


def _labels_np(ov, bs):
    K = ov.shape[0]
    labels1 = np.repeat(np.arange(K), bs)
    non = (ov == 0).astype(np.int64)
    excl = np.cumsum(non) - non
    cls2 = np.where(ov.astype(bool), np.arange(K), K + excl)
    labels2 = np.repeat(cls2, bs)
    return np.concatenate([labels1, labels2])


def _pair_rows(b):
    """Global row indices of block-pair b: blocks [b, 8+b]."""
    return np.concatenate([
        np.arange(b * BS, (b + 1) * BS),
        np.arange((8 + b) * BS, (8 + b + 1) * BS),
    ])


def _tile_to_ji(a):
    """Device tile [128, 2048] (col = m'*512 + i) -> [512 j, 512 i]."""
    return np.ascontiguousarray(
        a.reshape(128, 4, 512).transpose(1, 0, 2).reshape(512, 512))


def kernel(feats1, feats2, overlap_inds, bs):
    global _BUILT, _LAST_RESULTS
    bs = int(bs)
    feats1 = np.asarray(feats1, np.float32)
    feats2 = np.asarray(feats2, np.float32)
    ov = np.asarray(overlap_inds)
    assert feats1.shape == (2048, 512) and feats2.shape == (2048, 512)
    assert bs == BS and ov.shape == (8,)

    feats = np.concatenate([feats1, feats2])               # [N, F]
    labels = _labels_np(ov, bs)                            # [N]
    lblock = labels[::BS]                                  # [16]

    f8 = feats.astype(F8)

    in_maps = []
    for c in range(NCORES):
        rows = np.concatenate(
            [_pair_rows((c + PEER_OF_CHUNK[u]) % 8) for u in range(NCHUNK)])
        fq = f8[rows]                                      # [2560, 512]
        # [u, x, k, s, p] -> [p, k, u, s, x] (matches the SBUF/DRAM layout)
        arr = fq.reshape(NCHUNK, 512, 2, 2, 128).transpose(4, 2, 0, 3, 1)
        arr = np.ascontiguousarray(arr.reshape(128, NCHUNK * 2048))
        in_maps.append({"ft": arr})

    if _BUILT is None:
        _BUILT = _build_nc()
    nc = _BUILT

    try:
        res = run_bass_kernel_spmd(nc, in_maps, core_ids=list(range(NCORES)))
    except Exception:
        res = run_bass_kernel_spmd(nc, in_maps, core_ids=list(range(NCORES)))
    _LAST_RESULTS = res

    # ---- host assembly ----
    counts = np.bincount(labels)
    total_pos = float((counts[labels] - 1).sum())

    # exact possum via block sums (f64)
    feats64 = feats.astype(np.float64)
    Sblk = feats64.reshape(16, BS, F).sum(axis=1)          # [16, F]
    gblk = feats64 @ Sblk.T                                # [N, 16] exact
    gdiag = (feats64 * feats64).sum(axis=1)                # [N]

    # ship tiles in [j, i] orientation; shipg[(c, d)] = G^T between
    # j-rows of pair (c+d) and i-rows of core c's own pair, d in 1..3
    shipg = {}
    esum_dev = {}
    rmax_dev = {}
    for c in range(NCORES):
        out = res.results[c]
        for u in SHIP_CHUNKS:
            d = PEER_OF_CHUNK[u]
            shipg[(c, d)] = _tile_to_ji(
                out["ship"][SHIP_SLOT[u]].astype(np.float64))
        esum_dev[c] = out["esum"].astype(np.float64)[0]    # [512]
        rf = out["rmax"].astype(np.float64)                # [128, (2,512)]
        rmax_dev[c] = np.concatenate([rf[:, 0:512], rf[:, 512:1024]], axis=0)

    cnt_rows = np.zeros(N, np.float64)
    lossnum_rows = np.zeros(N, np.float64)
    need_refine = []

    idx512 = np.arange(512)
    for c in range(NCORES):
        rows = _pair_rows(c)                               # own 512 global rows
        paired = bool(lblock[c] == lblock[8 + c])
        fown = feats64[rows]
        t0 = fown @ fown.T                                 # exact own-pair G

        # --- negsum + max over the 7 peer pairs ---
        negsum = esum_dev[c].copy()                        # exp pair +4
        emax = rmax_dev[c].max(axis=0)                     # e-domain
        gmax = np.log(np.maximum(emax, 1e-30)) / TEMP      # [512] G-domain

        for d in (1, 2, 3):                                # own ships
            g = shipg[(c, d)]                              # [j(peer), i(own)]
            negsum += np.exp(TEMP * g).sum(axis=0)
            gmax = np.maximum(gmax, g.max(axis=0))
        for d in (1, 2, 3):                                # peer ships
            g = shipg[((c - d) % 8, d)]                    # [j(our rows), i(peer)]
            negsum += np.exp(TEMP * g).sum(axis=1)
            gmax = np.maximum(gmax, g.max(axis=1))

        if not paired:
            # partner block is negative: for i<256 partner j in [256,512)
            ep = np.exp(TEMP * t0)
            negsum[:256] += ep[256:, :256].sum(axis=0)
            negsum[256:] += ep[:256, 256:].sum(axis=0)
            gmax[:256] = np.maximum(gmax[:256], t0[256:, :256].max(axis=0))
            gmax[256:] = np.maximum(gmax[256:], t0[:256, 256:].max(axis=0))
        thr_g = gmax                                       # [512]

        # --- counts + margins from exact t0 ---
        diagmask = np.ones((512, 512), bool)
        diagmask[idx512, idx512] = False
        marg = np.abs(t0 - thr_g[None, :])
        above = t0 > thr_g[None, :]

        selfm = diagmask.copy()
        selfm[256:, :256] = False
        selfm[:256, 256:] = False
        cnt = (above & selfm).sum(axis=0).astype(np.float64)
        mmin = np.where(selfm, marg, np.inf).min(axis=0)
        if paired:
            crossm = np.zeros((512, 512), bool)
            crossm[256:, :256] = True
            crossm[:256, 256:] = True
            cnt += (above & crossm).sum(axis=0)
            mmin = np.minimum(mmin, np.where(crossm, marg, np.inf).min(axis=0))

        cnt_rows[rows] = cnt

        pw = 255.0 + (OTHERWEIGHT * 256.0 if paired else 0.0)
        b_self = np.where(idx512 < 256, c, 8 + c)
        possum = gblk[rows, b_self] - gdiag[rows]
        if paired:
            b_part = np.where(idx512 < 256, 8 + c, c)
            possum = possum + OTHERWEIGHT * gblk[rows, b_part]
        lossnum_rows[rows] = pw * np.log(negsum) - TEMP * possum

        need_refine.extend(rows[mmin < MARGIN])

    # exact recount of borderline rows (replicates reference ops, batched)
    if need_refine:
        idx = np.array(sorted(set(need_refine)), np.int64)
        g_ref = (feats[idx] @ feats.T).astype(np.float32)  # [R, N]
        sim = np.exp(g_ref * np.float32(TEMP)).astype(np.float32)
        for j, i in enumerate(idx):
            negm = labels != labels[i]
            mneg = sim[j, negm].max()
            posm = labels == labels[i]
            posm[i] = False
            cnt_rows[i] = float((sim[j, posm] > mneg).sum())

    acc = np.float32(cnt_rows.sum() / total_pos)
    loss = np.float32(lossnum_rows.sum() / total_pos)
    return acc, loss


# revision 40
# speedup vs baseline: 1.0215x; 1.0215x over previous
"""Trainium2 Bass kernel for nn_ContrastiveLoss (8-core data-parallel).

Transposed-tile (G^T) design with cross-core pair deduplication:
  Each core c owns row-blocks {c, 8+c} (512 rows).  G = feats@feats.T is
  symmetric, so each unordered block-pair tile is computed by exactly one
  core.  Core c computes G^T column-tiles [j-rows, own-i] for 4 peers:
    peers c+1, c+2, c+3 -> raw fp8 ship (host: exp-sums/maxes).  The
        tiles for peers c-1, c-2, c-3 come from THOSE peers' ships
        (the host transposes).
    peer c+4 -> ACT exp -> bf16 e-tile; per-i e-sums via ones-matmul
        into an accumulating PSUM row (exact f32); per-i maxes via a
        DVE fold tree -> rmax.  (Pair (c, c+4) is exp'd by both ends.)
  The own-pair 512x512 tile (positives + diagonal) is computed exactly
  on the host (f64), which also sharpens the count margins.
  fp8 DoubleRow matmuls; PE also does the ones-matmul partition sums.
Host: negsum/thr/count/margins/possum (exact block-sum trick)/loss;
borderline rows (|G - thr| < MARGIN) recounted exactly in f32.
"""

import os
import sys

sys.path.insert(0, "/opt/trn_rl_repo")
os.environ["BASS_NEVER_TRACE"] = "1"

from contextlib import ExitStack

import numpy as np
import ml_dtypes

import concourse.mybir as mybir
import concourse.tile as tile
from concourse import bacc
from concourse.bass_utils import run_bass_kernel_spmd

TEMP = 0.02
OTHERWEIGHT = 0.5

NCORES = 8
N = 4096
F = 512
BS = 256
NCHUNK = 5            # own, then 4 peer chunks
PEER_OF_CHUNK = [0, 4, 1, 2, 3]   # chunk u holds blocks of peer c+PEER_OF_CHUNK[u]
EXP_CHUNK = 1         # chunk index treated with on-device exp (peer +4)
SHIP_CHUNKS = (2, 3, 4)           # fp8 raw ships (peers +1, +2, +3)
SHIP_SLOT = {2: 0, 3: 1, 4: 2}    # chunk -> ship_d row
# (chunk, half, engine) processing ring: consumers alternate ACT/DVE so the
# 3-deep PSUM slot rotation never stalls on one engine; the exp tile rides
# in the first input batch so its slow output chains finish mid-kernel
SCHED = [(1, 0, 'A'), (2, 0, 'D'), (1, 1, 'A'), (2, 1, 'A'),
         (3, 1, 'D'), (3, 0, 'A'), (4, 1, 'D'), (4, 0, 'A')]
ESB_ENGINE = 'A'
MARGIN = 14.0         # G-domain refinement margin (fp8 matmul + fp8 ship noise)
WARM_MMS = 10         # PE p-state warmup matmuls

F8 = ml_dtypes.float8_e4m3
BF16 = ml_dtypes.bfloat16

_BUILT = None
_LAST_RESULTS = None


def _build_nc():
    f32 = mybir.dt.float32
    bf16 = mybir.dt.bfloat16
    fp8 = mybir.dt.float8e4
    Exp = mybir.ActivationFunctionType.Exp
    Copy = mybir.ActivationFunctionType.Copy
    DR = mybir.MatmulPerfMode.DoubleRow
    MAX = mybir.AluOpType.max

    nc = bacc.Bacc("TRN2", target_bir_lowering=False, debug=False)
    # DRAM input laid out exactly like SBUF: [p, k*(NCHUNK*1024) + u*1024 + sx]
    ft_d = nc.dram_tensor("ft", [128, NCHUNK * 2048], fp8, kind="ExternalInput")
    ship_d = nc.dram_tensor("ship", [3, 128, 2048], fp8, kind="ExternalOutput")
    rmax_d = nc.dram_tensor("rmax", [128, 1024], bf16, kind="ExternalOutput")
    esum_d = nc.dram_tensor("esum", [1, 512], f32, kind="ExternalOutput")

    with tile.TileContext(nc) as tc, ExitStack() as ctx:
        ftp = ctx.enter_context(tc.tile_pool(name="ft", bufs=1))
        sp = ctx.enter_context(tc.tile_pool(name="sp", bufs=1))
        psp = ctx.enter_context(tc.tile_pool(name="ps", bufs=1, space="PSUM"))

        ft_t = ftp.tile([128, NCHUNK * 2048], fp8, name="ft", tag="ft")
        # col = u*2048 + x2*1024 + k*512 + s*256 + xq  (x = x2*256 + xq).
        # x2 outermost-within-chunk makes "chunk0 + first half of chunk1"
        # one contiguous DMA, so the first matmuls start a transfer earlier.
        ftg = ft_t[:].rearrange(
            "p (u x2 k s xq) -> p u x2 k s xq", u=NCHUNK, x2=2, k=2, s=2)
        ftrh = ft_t[:].rearrange(
            "p (u x2 k s xq) -> p u k s x2 xq", u=NCHUNK, x2=2, k=2, s=2)
        # batches: [own + e1-x2=0], [s1], [e1-x2=1], [s2], [s3]
        for lo, hi in [(0, 3072), (4096, 6144), (3072, 4096),
                       (6144, 8192), (8192, 10240)]:
            nc.sync.dma_start(ft_t[:, lo:hi], ft_d.ap()[:, lo:hi])

        eship = sp.tile([128, 3 * 2048], fp8, name="eship", tag="eship")
        et = sp.tile([128, 2048], bf16, name="et", tag="et")
        rfold = sp.tile([128, 1024], bf16, name="rfold", tag="rfold")
        esb = sp.tile([1, 512], f32, name="esb", tag="esb")
        ones_t = sp.tile([128, 1], bf16, name="ones", tag="ones")
        dummy = sp.tile([128, 1], bf16, name="dummy", tag="dummy")
        nc.vector.memset(ones_t[:], 1.0)

        # Hoist ACT table loads off the critical path: touch Exp and Copy
        # tables with tiny dummy activations before any real dependency.
        nc.scalar.activation(dummy[:], ones_t[:], Exp)
        nc.scalar.activation(dummy[:], ones_t[:], Copy)

        # PSUM: 3 rotating half-slots (6 banks) + esum row (1) + warm (1)
        ps_t = [psp.tile([128, 1024], f32, name=f"ps{i}", tag=f"ps{i}")
                for i in range(3)]
        es_ps = psp.tile([1, 512], f32, name="esps", tag="esps")
        warm_ps = psp.tile([128, 512], f32, name="warmps", tag="warmps")

        # PE p-state warmup: PE busy from t~0 so real matmuls hit full clock.
        warm = sp.tile([128, 256], fp8, name="warm", tag="warm")
        nc.vector.memset(warm[:], 0.0)
        wr = warm[:].rearrange("p (s x) -> p s x", s=2)
        for _ in range(WARM_MMS):
            nc.tensor.matmul(
                warm_ps[0:128, 0:128], wr[:, :, 0:128], wr[:, :, 0:128],
                start=True, stop=True, perf_mode=DR,
            )

        pend_ones = []    # deferred ones-MM slices (avoid PE head-of-line stall)
        ones_emitted = [0]

        def flush_ones():
            for sl in pend_ones:
                ones_emitted[0] += 1
                nc.tensor.matmul(
                    es_ps[:], ones_t[:], sl,
                    start=(ones_emitted[0] == 1),
                    stop=(ones_emitted[0] == 4),
                    skip_group_check=True,
                )
            pend_ones.clear()

        ship_halves_done = {u: 0 for u in SHIP_CHUNKS}
        for pos, (t, h, eng) in enumerate(SCHED):
            ps = ps_t[pos % 3]
            for g in (2 * h, 2 * h + 1):
                for k in range(2):
                    nc.tensor.matmul(
                        ps[:, (g % 2) * 512:(g % 2 + 1) * 512],
                        ftg[:, t, g // 2, k, :,
                            (g % 2) * 128:(g % 2) * 128 + 128],
                        ftrh[:, 0, k, :, :, :],
                        start=(k == 0),
                        stop=(k == 1),
                        perf_mode=DR,
                    )
            flush_ones()
            if t == EXP_CHUNK:
                with nc.allow_low_precision(reason="bf16 e; host-tol ok"):
                    nc.scalar.activation(
                        et[:, h * 1024:(h + 1) * 1024], ps[:], Exp, scale=TEMP)
                for g in (2 * h, 2 * h + 1):
                    pend_ones.append(et[:, g * 512:(g + 1) * 512])
                # per-half max fold (bf16 2x on DVE)
                with nc.allow_low_precision(reason="bf16 max; host refine"):
                    nc.vector.tensor_tensor(
                        rfold[:, h * 512:(h + 1) * 512],
                        et[:, h * 1024:h * 1024 + 512],
                        et[:, h * 1024 + 512:h * 1024 + 1024], op=MAX)
            else:
                slot = SHIP_SLOT[t]
                dst = eship[:, slot * 2048 + h * 1024:
                            slot * 2048 + (h + 1) * 1024]
                if eng == 'A':
                    nc.scalar.activation(dst, ps[:], Copy)
                elif eng == 'D':
                    nc.vector.tensor_copy(dst, ps[:])
                else:  # 'S': split the copy across both engines
                    nc.vector.tensor_copy(dst[:, 0:512], ps[:, 0:512])
                    nc.scalar.activation(dst[:, 512:1024], ps[:, 512:1024], Copy)
                ship_halves_done[t] += 1
                if t == 4:
                    # last ship tile: per-half DMA so the final transfer is small
                    nc.sync.dma_start(
                        ship_d.ap()[slot, :, h * 1024:(h + 1) * 1024], dst)
                elif ship_halves_done[t] == 2:
                    nc.sync.dma_start(
                        ship_d.ap()[slot],
                        eship[:, slot * 2048:(slot + 1) * 2048])
        flush_ones()
        if ESB_ENGINE == 'A':
            nc.scalar.activation(esb[:], es_ps[:], Copy)
        else:
            nc.vector.tensor_copy(esb[:], es_ps[:])
        # epilogue DMAs in expected-readiness order (HWDGE is in-order; a
        # slow-dep DMA issued early head-of-line-blocks everything after it)
        nc.sync.dma_start(rmax_d.ap()[:], rfold[:])
        nc.sync.dma_start(esum_d.ap()[:], esb[:])

    nc.compile()
    return nc


def _labels_np(ov, bs):
    K = ov.shape[0]
    labels1 = np.repeat(np.arange(K), bs)
    non = (ov == 0).astype(np.int64)
    excl = np.cumsum(non) - non
    cls2 = np.where(ov.astype(bool), np.arange(K), K + excl)
    labels2 = np.repeat(cls2, bs)
    return np.concatenate([labels1, labels2])


def _pair_rows(b):
    """Global row indices of block-pair b: blocks [b, 8+b]."""
    return np.concatenate([
        np.arange(b * BS, (b + 1) * BS),
        np.arange((8 + b) * BS, (8 + b + 1) * BS),
    ])


def _tile_to_ji(a):
    """Device tile [128, 2048] (col = m'*512 + i) -> [512 j, 512 i]."""
    return np.ascontiguousarray(
        a.reshape(128, 4, 512).transpose(1, 0, 2).reshape(512, 512))


def kernel(feats1, feats2, overlap_inds, bs):
    global _BUILT, _LAST_RESULTS
    bs = int(bs)
    feats1 = np.asarray(feats1, np.float32)
    feats2 = np.asarray(feats2, np.float32)
    ov = np.asarray(overlap_inds)
    assert feats1.shape == (2048, 512) and feats2.shape == (2048, 512)
    assert bs == BS and ov.shape == (8,)

    feats = np.concatenate([feats1, feats2])               # [N, F]
    labels = _labels_np(ov, bs)                            # [N]
    lblock = labels[::BS]                                  # [16]

    f8 = feats.astype(F8)

    in_maps = []
    for c in range(NCORES):
        rows = np.concatenate(
            [_pair_rows((c + PEER_OF_CHUNK[u]) % 8) for u in range(NCHUNK)])
        fq = f8[rows]                                      # [2560, 512]
        # [u, x2, xq, k, s, p] -> [p, u, x2, k, s, xq] (SBUF/DRAM layout)
        arr = fq.reshape(NCHUNK, 2, 256, 2, 2, 128).transpose(5, 0, 1, 3, 4, 2)
        arr = np.ascontiguousarray(arr.reshape(128, NCHUNK * 2048))
        in_maps.append({"ft": arr})

    if _BUILT is None:
        _BUILT = _build_nc()
    nc = _BUILT

    try:
        res = run_bass_kernel_spmd(nc, in_maps, core_ids=list(range(NCORES)))
    except Exception:
        res = run_bass_kernel_spmd(nc, in_maps, core_ids=list(range(NCORES)))
    _LAST_RESULTS = res

    # ---- host assembly ----
    counts = np.bincount(labels)
    total_pos = float((counts[labels] - 1).sum())

    # exact possum via block sums (f64)
    feats64 = feats.astype(np.float64)
    Sblk = feats64.reshape(16, BS, F).sum(axis=1)          # [16, F]
    gblk = feats64 @ Sblk.T                                # [N, 16] exact
    gdiag = (feats64 * feats64).sum(axis=1)                # [N]

    # ship tiles in [j, i] orientation; shipg[(c, d)] = G^T between
    # j-rows of pair (c+d) and i-rows of core c's own pair, d in 1..3
    shipg = {}
    esum_dev = {}
    rmax_dev = {}
    for c in range(NCORES):
        out = res.results[c]
        for u in SHIP_CHUNKS:
            d = PEER_OF_CHUNK[u]
            shipg[(c, d)] = _tile_to_ji(
                out["ship"][SHIP_SLOT[u]].astype(np.float64))
        esum_dev[c] = out["esum"].astype(np.float64)[0]    # [512]
        rf = out["rmax"].astype(np.float64)                # [128, (2,512)]
        rmax_dev[c] = np.concatenate([rf[:, 0:512], rf[:, 512:1024]], axis=0)

    cnt_rows = np.zeros(N, np.float64)
    lossnum_rows = np.zeros(N, np.float64)
    need_refine = []

    idx512 = np.arange(512)
    for c in range(NCORES):
        rows = _pair_rows(c)                               # own 512 global rows
        paired = bool(lblock[c] == lblock[8 + c])
        fown = feats64[rows]
        t0 = fown @ fown.T                                 # exact own-pair G

        # --- negsum + max over the 7 peer pairs ---
        negsum = esum_dev[c].copy()                        # exp pair +4
        emax = rmax_dev[c].max(axis=0)                     # e-domain
        gmax = np.log(np.maximum(emax, 1e-30)) / TEMP      # [512] G-domain

        for d in (1, 2, 3):                                # own ships
            g = shipg[(c, d)]                              # [j(peer), i(own)]
            negsum += np.exp(TEMP * g).sum(axis=0)
            gmax = np.maximum(gmax, g.max(axis=0))
        for d in (1, 2, 3):                                # peer ships
            g = shipg[((c - d) % 8, d)]                    # [j(our rows), i(peer)]
            negsum += np.exp(TEMP * g).sum(axis=1)
            gmax = np.maximum(gmax, g.max(axis=1))

        if not paired:
            # partner block is negative: for i<256 partner j in [256,512)
            ep = np.exp(TEMP * t0)
            negsum[:256] += ep[256:, :256].sum(axis=0)
            negsum[256:] += ep[:256, 256:].sum(axis=0)
            gmax[:256] = np.maximum(gmax[:256], t0[256:, :256].max(axis=0))
            gmax[256:] = np.maximum(gmax[256:], t0[:256, 256:].max(axis=0))
        thr_g = gmax                                       # [512]

        # --- counts + margins from exact t0 ---
        diagmask = np.ones((512, 512), bool)
        diagmask[idx512, idx512] = False
        marg = np.abs(t0 - thr_g[None, :])
        above = t0 > thr_g[None, :]

        selfm = diagmask.copy()
        selfm[256:, :256] = False
        selfm[:256, 256:] = False
        cnt = (above & selfm).sum(axis=0).astype(np.float64)
        mmin = np.where(selfm, marg, np.inf).min(axis=0)
        if paired:
            crossm = np.zeros((512, 512), bool)
            crossm[256:, :256] = True
            crossm[:256, 256:] = True
            cnt += (above & crossm).sum(axis=0)
            mmin = np.minimum(mmin, np.where(crossm, marg, np.inf).min(axis=0))

        cnt_rows[rows] = cnt

        pw = 255.0 + (OTHERWEIGHT * 256.0 if paired else 0.0)
        b_self = np.where(idx512 < 256, c, 8 + c)
        possum = gblk[rows, b_self] - gdiag[rows]
        if paired:
            b_part = np.where(idx512 < 256, 8 + c, c)
            possum = possum + OTHERWEIGHT * gblk[rows, b_part]
        lossnum_rows[rows] = pw * np.log(negsum) - TEMP * possum

        need_refine.extend(rows[mmin < MARGIN])

    # exact recount of borderline rows (replicates reference ops, batched)
    if need_refine:
        idx = np.array(sorted(set(need_refine)), np.int64)
        g_ref = (feats[idx] @ feats.T).astype(np.float32)  # [R, N]
        sim = np.exp(g_ref * np.float32(TEMP)).astype(np.float32)
        for j, i in enumerate(idx):
            negm = labels != labels[i]
            mneg = sim[j, negm].max()
            posm = labels == labels[i]
            posm[i] = False
            cnt_rows[i] = float((sim[j, posm] > mneg).sum())

    acc = np.float32(cnt_rows.sum() / total_pos)
    loss = np.float32(lossnum_rows.sum() / total_pos)
    return acc, loss
